# revision 11
# baseline (speedup 1.0000x reference)
"""Causal self-attention (B=4, T=2048, C=1024, H=16) on 8 Trainium2 NeuronCores.

Sharding: 8 cores = 4 batches x 2 head-groups. Core c handles batch c//2 and
heads 8*(c%2) .. 8*(c%2)+8 (512 of the 1024 channels). Each core computes the
QKV projection for its channels over its batch's 2048 tokens, flash-style
causal attention for its 8 heads, and a partial output projection over its
512 c_proj input rows. The host sums the two partials per batch and adds the
bias terms (b_proj plus the b_v contribution, which is w_proj @ b_v because
softmax rows sum to one; b_k shifts every logit in a row equally so softmax
drops it; b_q and the 1/sqrt(hd) scale are folded into the q weights host-side).

All matmul inputs are bf16 (PSUM accumulation stays fp32). The schedule is
chunk-pipelined and latency-oriented: attention is ScalarE(exp)-bound per
tile, so the per-chunk loop emits, for each of the 4 head pairs, the j-loop
[S-pair row-tiled at partition bases 0/64, exp pair, diagonal mask on Pool,
PV pair one step behind] with QKV matmuls for the NEXT chunk and proj matmuls
for the PREVIOUS chunk hand-interleaved between j-steps to fill the PE gaps
left while ScalarE catches up. Band tiles (partially-masked diagonal blocks)
compute exp and the PV matmul only on the live suffix [z:512]; the masked
prefix is never written or read, so no zeroing and no persistent band
buffers are needed.

Softmax denominators ride in V's 65th column (ones), so ys row 64 is the
denominator; a [1,512] VectorE reciprocal and a K=1 ones-outer-product
matmul broadcast 1/denom across the 64 head channels for the normalize.

This container's walrus accepts only one hardware wait slot per instruction,
so after Tile scheduling we split multi-wait sync_info into standalone
EventSemaphore waits (_split_multiwaits).
"""

import sys

if '/opt/trn_rl_repo' not in sys.path:
    sys.path.insert(0, '/opt/trn_rl_repo')

import numpy as np

B, T, C, H = 4, 2048, 1024, 16
HD = C // H            # 64
HPC = 8                # heads per core
CL = HPC * HD          # 512 local channels
NCORES = 8
NQ = T // 512          # 4 q-chunks of 512 tokens
NT = T // 128          # 16 token-blocks

_cache = {}


def _split_multiwaits(nc, max_waits=1):
    import concourse.mybir as mybir
    n = 0
    ctr = [0]
    for fn in nc.m.functions:
        for bb in fn.blocks:
            out = []
            for inst in bb.instructions:
                si = inst.sync_info
                if si is not None and si.on_wait and len(si.on_wait) > max_waits:
                    waits = list(si.on_wait)
                    head, tail = waits[:-max_waits], waits[-max_waits:]
                    for w in head:
                        ctr[0] += 1
                        out.append(mybir.InstEventSemaphore(
                            name=f"wsplit-{ctr[0]}",
                            engine=inst.engine,
                            ins=[], outs=[],
                            sync_info=mybir.SyncInfo(on_wait=[w], on_update=[]),
                        ))
                    inst.sync_info = mybir.SyncInfo(
                        on_wait=tail, on_update=list(si.on_update))
                    n += 1
                out.append(inst)
            bb.instructions[:] = out
    return n


def _build(reps=1, phases="qkv,attn,norm,proj"):
    phases = set(phases.split(","))
    import contextlib
    import concourse.bass as bass
    import concourse.mybir as mybir
    import concourse.tile as tile

    f32 = mybir.dt.float32
    f32r = mybir.dt.float32r
    bf16 = mybir.dt.bfloat16
    Act = mybir.ActivationFunctionType
    Alu = mybir.AluOpType

    nc = bass.Bass()

    xT_d = nc.dram_tensor("xT", [C, T], bf16, kind="ExternalInput")
    wqk_d = nc.dram_tensor("wqk", [C, 2 * CL], bf16, kind="ExternalInput")
    wv_d = nc.dram_tensor("wv", [C, CL], bf16, kind="ExternalInput")
    bq_d = nc.dram_tensor("bq", [128, 4], f32, kind="ExternalInput")
    wp_d = nc.dram_tensor("wp", [CL, C], bf16, kind="ExternalInput")
    mask_d = nc.dram_tensor("mask", [128, 128], bf16, kind="ExternalInput")
    out_d = nc.dram_tensor("out", [T, C], f32, kind="ExternalOutput")

    with tile.TileContext(nc) as tc:
        with tc.tile_pool(name="persist", bufs=1) as persist, \
             tc.tile_pool(name="xp", bufs=2) as xp, \
             tc.tile_pool(name="pt", bufs=6) as ptpool, \
             tc.tile_pool(name="ysb", bufs=4) as ypool, \
             tc.tile_pool(name="outst", bufs=2) as opool, \
             tc.tile_pool(name="rst", bufs=2) as rpool, \
             tc.tile_pool(name="psmm", bufs=2, space="PSUM") as psmm, \
             tc.tile_pool(name="psst", bufs=3, space="PSUM") as psst, \
             tc.tile_pool(name="psy", bufs=2, space="PSUM") as psy, \
             tc.tile_pool(name="psrep", bufs=1, space="PSUM") as psrep:

            with (tc.For_i(0, reps, 1) if reps > 1 else contextlib.nullcontext()):
                wqk_sb = persist.tile([128, 8, 2 * CL], bf16, tag="wqk")
                wv_sb = persist.tile([128, 8, CL], bf16, tag="wv")
                wp_sb = persist.tile([128, 4, C], bf16, tag="wp")
                qkT = persist.tile([128, 8, T], bf16, tag="qkT")
                V = persist.tile([128, NT, HPC, HD + 1], bf16, tag="V")
                yN = persist.tile([128, 4, T], bf16, tag="yN")
                bq_sb = persist.tile([128, 4], f32, tag="bq")
                mask_sb = persist.tile([128, 128], bf16, tag="mask")
                ones64 = persist.tile([1, 64], bf16, tag="ones")

                nc.sync.dma_start(bq_sb[:], bq_d[:])
                nc.sync.dma_start(mask_sb[:], mask_d[:])
                nc.sync.dma_start(
                    wqk_sb[:], wqk_d.rearrange("(j p) m -> p j m", p=128))
                nc.sync.dma_start(
                    wv_sb[:], wv_d.rearrange("(j p) m -> p j m", p=128))
                nc.sync.dma_start(
                    wp_sb[:], wp_d.rearrange("(mq p) oc -> p mq oc", p=128))
                nc.vector.memset(ones64[:], 1.0)
                nc.vector.memset(V[:, :, :, HD:HD + 1], 1.0)

                xq = [None] * NQ

                def load_x(c):
                    xq[c] = xp.tile([128, 8, 512], bf16, tag="x",
                                    name=f"x{c}")
                    nc.sync.dma_start(
                        xq[c][:],
                        xT_d[:, c * 512:c * 512 + 512]
                        .rearrange("(j p) n -> p j n", p=128))

                def qkv_part_ops(c, i):
                    """One quarter of chunk c's QKV as a list of closures,
                    each emitting one instruction (27 per part). The three
                    accumulation chains (q/k blocks i and i+4, V block i)
                    are interleaved so consecutive PE matmuls hit different
                    PSUM banks (same-bank accumulation back-pressure costs
                    ~100 ns/matmul otherwise)."""
                    cs = c * 512
                    tb = c * 4 + i
                    pss = {}

                    def mk_qk(m, j):
                        def emit():
                            if j == 0:
                                pss[m] = psmm.tile([128, 512], f32,
                                                   tag="mm", name="mmqk")
                            nc.tensor.matmul(
                                pss[m][:],
                                lhsT=wqk_sb[:, j, m * 128:m * 128 + 128],
                                rhs=xq[c][:, j, :],
                                start=(j == 0), stop=(j == 7))
                        return emit

                    def mk_vmm(j):
                        def emit():
                            if j == 0:
                                pss['v'] = psmm.tile([128, 512], f32,
                                                     tag="mm", name="mmv")
                            nc.tensor.matmul(
                                pss['v'][:],
                                lhsT=xq[c][:, j, i * 128:i * 128 + 128],
                                rhs=wv_sb[:, j, :],
                                start=(j == 0), stop=(j == 7))
                        return emit

                    def mk_evac(m):
                        def emit():
                            if m == 'v':
                                nc.vector.tensor_copy(
                                    V[:, tb, :, 0:HD],
                                    pss['v'][:].rearrange(
                                        "p (h d) -> p h d", h=HPC))
                            elif m < 4:
                                nc.vector.tensor_scalar_add(
                                    qkT[:, m, cs:cs + 512], pss[m][:],
                                    bq_sb[:, m:m + 1])
                            else:
                                nc.vector.tensor_copy(
                                    qkT[:, m, cs:cs + 512], pss[m][:])
                        return emit

                    ops = []
                    for j in range(8):
                        ops += [mk_qk(i, j), mk_qk(i + 4, j), mk_vmm(j)]
                    ops += [mk_evac(i), mk_evac(i + 4), mk_evac('v')]
                    return ops

                def proj_ops(c):
                    """Output projection for chunk c as closures (one
                    instruction each); accumulation chains interleaved in
                    groups of 3 to alternate PSUM banks."""
                    tiles = [(o, t) for o in range(2)
                             for t in range(c * 4, c * 4 + 4)]
                    pss = {}

                    def mk_mm(o, t, mq):
                        def emit():
                            if mq == 0:
                                pss[(o, t)] = psmm.tile([128, 512], f32,
                                                        tag="mm", name="mmpj")
                            nc.tensor.matmul(
                                pss[(o, t)][:],
                                lhsT=yN[:, mq, t * 128:t * 128 + 128],
                                rhs=wp_sb[:, mq, o * 512:o * 512 + 512],
                                start=(mq == 0), stop=(mq == 3))
                        return emit

                    def mk_evac(o, t):
                        def emit():
                            ost = opool.tile([128, 512], f32, tag="outst",
                                             name="ost")
                            nc.vector.tensor_copy(ost[:], pss[(o, t)][:])
                            nc.sync.dma_start(
                                out_d[t * 128:t * 128 + 128,
                                      o * 512:o * 512 + 512],
                                ost[:])
                        return emit

                    ops = []
                    for g in range(0, len(tiles), 3):
                        grp = tiles[g:g + 3]
                        for mq in range(4):
                            ops += [mk_mm(o, t, mq) for (o, t) in grp]
                        ops += [mk_evac(o, t) for (o, t) in grp]
                    return ops

                def attn_pair(m, c, fill, sl, pending):
                    """Attention for head pair (2m, 2m+1), q-chunk c.
                    Pops filler ops per j-step (rate set so the fill list
                    drains evenly over the c-iteration) to fill PE gaps.
                    Deferred norm ops from the previous pair are emitted at
                    j==1, by which point their reciprocal inputs are long
                    done, so the rep matmuls don't stall the PE queue."""
                    cs = c * 512
                    nj = 4 * c + 4
                    ys = [psy.tile([65, 512], f32, tag="y",
                                   name=f"ys{m}{c}{s}") for s in (0, 1)]
                    prev = None
                    for j in range(nj):
                        jrel = j - 4 * c
                        z = 128 * jrel if jrel > 0 else 0
                        if j in (1, 2) and pending:
                            pending.pop(0)()
                        k = min(8, -(-len(fill) // max(sl[0], 1)))
                        sl[0] -= 1
                        for _ in range(k):
                            if fill:
                                fill.pop(0)()
                        pss = []
                        for s in (0, 1):
                            pb = 64 * s
                            ps_st = psst.tile([128, 512], f32, tag="st",
                                              name="st")
                            nc.tensor.matmul(
                                ps_st[:, z:512],
                                lhsT=qkT[pb:pb + 64, 4 + m,
                                         j * 128:j * 128 + 128],
                                rhs=qkT[pb:pb + 64, m, cs + z:cs + 512],
                                start=True, stop=True)
                            pss.append(ps_st)
                        pts = []
                        for s in (0, 1):
                            pt = ptpool.tile([128, 512], bf16, tag="pt",
                                             name="pt")
                            nc.scalar.activation(
                                pt[:, z:512], pss[s][:, z:512], Act.Exp)
                            pts.append(pt)
                        if jrel >= 0:
                            for s in (0, 1):
                                nc.gpsimd.tensor_tensor(
                                    pts[s][:, z:z + 128],
                                    pts[s][:, z:z + 128],
                                    mask_sb[:], Alu.mult)
                        if prev is not None:
                            pj, ppts, pz = prev
                            for s in (0, 1):
                                nc.tensor.matmul(
                                    ys[s][:, pz:512],
                                    lhsT=V[:, pj, 2 * m + s, :],
                                    rhs=ppts[s][:, pz:512],
                                    start=(pj == 0), stop=False)
                        prev = (j, pts, z)
                    pj, ppts, pz = prev
                    for s in (0, 1):
                        nc.tensor.matmul(
                            ys[s][:, pz:512],
                            lhsT=V[:, pj, 2 * m + s, :],
                            rhs=ppts[s][:, pz:512],
                            start=(pj == 0), stop=True)
                    ycs = []
                    for s in (0, 1):
                        y_c = ypool.tile([65, 512], f32, tag="ysb",
                                         name=f"y{m}{c}{s}")
                        nc.vector.tensor_copy(y_c[:], ys[s][:])
                        ycs.append(y_c)
                    return ycs

                def norm_pair(m, c, ycs):
                    """Reciprocals emitted eagerly (VectorE-only chain);
                    returns two closures (rep matmul + multiply per head)
                    to emit later, when the reciprocals are long done, so
                    the rep matmuls never stall the in-order PE queue."""
                    cs = c * 512
                    rTs = []
                    for s in (0, 1):
                        rT = rpool.tile([1, 512], bf16, tag="rT",
                                        name="rT")
                        with nc.allow_low_precision(
                                reason="bf16 1/denom; 0.4% scale noise ok"):
                            nc.vector.reciprocal(rT[:], ycs[s][64:65, :])
                        rTs.append(rT)

                    def mk(s):
                        def emit():
                            ps_rep = psrep.tile([64, 512], f32, tag="rep",
                                                name="rep")
                            nc.tensor.matmul(
                                ps_rep[:], lhsT=ones64[:],
                                rhs=rTs[s][:], start=True, stop=True)
                            nc.vector.tensor_tensor(
                                yN[64 * s:64 * s + 64, m, cs:cs + 512],
                                ycs[s][0:64, :], ps_rep[:], Alu.mult)
                        return emit
                    return [mk(0), mk(1)]

                # ---------------- schedule ----------------
                if "qkv" in phases:
                    load_x(0)
                    for op in [o for i in range(4)
                               for o in qkv_part_ops(0, i)]:
                        op()

                pending = []
                for c in range(NQ):
                    fill = []
                    if "qkv" in phases and c + 1 < NQ:
                        load_x(c + 1)
                        for i in range(4):
                            fill += qkv_part_ops(c + 1, i)
                    if "proj" in phases and c >= 1:
                        fill += proj_ops(c - 1)
                    if "attn" in phases:
                        sl = [4 * (4 * c + 4)]
                        for m in range(4):
                            ycs = attn_pair(m, c, fill, sl, pending)
                            if "norm" in phases:
                                pending += norm_pair(m, c, ycs)
                    for op in pending:
                        op() if c == NQ - 1 else None
                    if c == NQ - 1:
                        pending = []
                    for op in fill:
                        op()
                if "proj" in phases:
                    for op in proj_ops(NQ - 1):
                        op()

    nsplit = _split_multiwaits(nc)
    return nc, nsplit


def _prep_inputs(x, w_attn, b_attn, w_proj):
    """Per-core input maps. Core c: batch c//2, head-group c%2."""
    import ml_dtypes
    bf16 = ml_dtypes.bfloat16
    x = np.ascontiguousarray(x, dtype=np.float32)
    w_attn = np.asarray(w_attn, dtype=np.float32)
    b_attn = np.asarray(b_attn, dtype=np.float32)
    w_proj = np.asarray(w_proj, dtype=np.float32)
    scale = 1.0 / np.sqrt(HD)

    mask = (np.arange(128)[:, None] <= np.arange(128)[None, :]).astype(bf16)

    in_maps = []
    for core in range(NCORES):
        b = core // 2
        g = core % 2
        gc = CL * g
        wq = w_attn[gc:gc + CL, :] * scale          # [512, 1024]
        wk = w_attn[C + gc:C + gc + CL, :]
        wv = w_attn[2 * C + gc:2 * C + gc + CL, :]
        bq = b_attn[gc:gc + CL] * scale
        in_maps.append({
            "xT": np.ascontiguousarray(x[b].T.astype(bf16)),
            "wqk": np.ascontiguousarray(
                np.concatenate([wq.T, wk.T], axis=1).astype(bf16)),
            "wv": np.ascontiguousarray(wv.T.astype(bf16)),
            "bq": np.ascontiguousarray(bq.reshape(4, 128).T),
            "wp": np.ascontiguousarray(
                w_proj[:, gc:gc + CL].T.astype(bf16)),
            "mask": mask,
        })
    return in_maps


def _run(in_maps, reps=1):
    from concourse.bass_utils import run_bass_kernel_spmd
    key = reps
    if key not in _cache:
        _cache[key] = _build(reps)
    nc, _ = _cache[key]
    return run_bass_kernel_spmd(nc, in_maps, list(range(NCORES)))


def kernel(x, w_attn, b_attn, w_proj, b_proj):
    x = np.asarray(x, dtype=np.float32)
    w_attn = np.asarray(w_attn, dtype=np.float32)
    b_attn = np.asarray(b_attn, dtype=np.float32)
    w_proj = np.asarray(w_proj, dtype=np.float32)
    b_proj = np.asarray(b_proj, dtype=np.float32)

    in_maps = _prep_inputs(x, w_attn, b_attn, w_proj)
    res = _run(in_maps).results

    # host-side unshard: sum the two head-group partials per batch and add
    # the bias terms (b_proj + w_proj @ b_v; softmax rows sum to 1).
    bv = b_attn[2 * C:]
    const = (w_proj @ bv + b_proj).astype(np.float32)
    out = np.empty((B, T, C), dtype=np.float32)
    for b in range(B):
        out[b] = res[2 * b]["out"] + res[2 * b + 1]["out"] + const
    return out


# revision 12
# speedup vs baseline: 1.2596x; 1.2596x over previous
"""Causal self-attention (B=4, T=2048, C=1024, H=16) on 8 Trainium2 NeuronCores.

Sharding: 8 cores = 4 batches x 2 head-groups. Core c handles batch c//2 and
heads 8*(c%2) .. 8*(c%2)+8 (512 of the 1024 channels). Each core computes the
QKV projection for its channels over its batch's 2048 tokens, flash-style
causal attention for its 8 heads, and a partial output projection over its
512 c_proj input rows. The host sums the two partials per batch and adds the
bias terms (b_proj plus the b_v contribution, which is w_proj @ b_v because
softmax rows sum to one; b_k shifts every logit in a row equally so softmax
drops it; b_q and the 1/sqrt(hd) scale are folded into the q weights host-side).

All matmul inputs are bf16 (PSUM accumulation stays fp32). The schedule is
chunk-pipelined and latency-oriented: attention is ScalarE(exp)-bound per
tile, so the per-chunk loop emits, for each of the 4 head pairs, the j-loop
[S-pair row-tiled at partition bases 0/64, exp pair, diagonal mask on Pool,
PV pair one step behind] with QKV matmuls for the NEXT chunk and proj matmuls
for the PREVIOUS chunk hand-interleaved between j-steps to fill the PE gaps
left while ScalarE catches up. Band tiles (partially-masked diagonal blocks)
compute exp and the PV matmul only on the live suffix [z:512]; the masked
prefix is never written or read, so no zeroing and no persistent band
buffers are needed.

Softmax denominators ride in V's 65th column (ones), so ys row 64 is the
denominator; a [1,512] VectorE reciprocal and a K=1 ones-outer-product
matmul broadcast 1/denom across the 64 head channels for the normalize.

This container's walrus accepts only one hardware wait slot per instruction,
so after Tile scheduling we split multi-wait sync_info into standalone
EventSemaphore waits (_split_multiwaits).
"""

import sys

if '/opt/trn_rl_repo' not in sys.path:
    sys.path.insert(0, '/opt/trn_rl_repo')

import numpy as np

B, T, C, H = 4, 2048, 1024, 16
HD = C // H            # 64
HPC = 8                # heads per core
CL = HPC * HD          # 512 local channels
NCORES = 8
NQ = T // 512          # 4 q-chunks of 512 tokens
NT = T // 128          # 16 token-blocks

_cache = {}


def _split_multiwaits(nc, max_waits=1):
    import concourse.mybir as mybir
    n = 0
    ctr = [0]
    for fn in nc.m.functions:
        for bb in fn.blocks:
            out = []
            for inst in bb.instructions:
                si = inst.sync_info
                if si is not None and si.on_wait and len(si.on_wait) > max_waits:
                    waits = list(si.on_wait)
                    head, tail = waits[:-max_waits], waits[-max_waits:]
                    for w in head:
                        ctr[0] += 1
                        out.append(mybir.InstEventSemaphore(
                            name=f"wsplit-{ctr[0]}",
                            engine=inst.engine,
                            ins=[], outs=[],
                            sync_info=mybir.SyncInfo(on_wait=[w], on_update=[]),
                        ))
                    inst.sync_info = mybir.SyncInfo(
                        on_wait=tail, on_update=list(si.on_update))
                    n += 1
                out.append(inst)
            bb.instructions[:] = out
    return n


def _build(reps=1, phases="qkv,attn,norm,proj"):
    phases = set(phases.split(","))
    import contextlib
    import concourse.bass as bass
    import concourse.mybir as mybir
    import concourse.tile as tile

    f32 = mybir.dt.float32
    f32r = mybir.dt.float32r
    bf16 = mybir.dt.bfloat16
    Act = mybir.ActivationFunctionType
    Alu = mybir.AluOpType

    nc = bass.Bass()

    xT_d = nc.dram_tensor("xT", [C, T], bf16, kind="ExternalInput")
    wqk_d = nc.dram_tensor("wqk", [C, 2 * CL], bf16, kind="ExternalInput")
    wv_d = nc.dram_tensor("wv", [C, CL], bf16, kind="ExternalInput")
    bq_d = nc.dram_tensor("bq", [128, 4], f32, kind="ExternalInput")
    wp_d = nc.dram_tensor("wp", [CL, C], bf16, kind="ExternalInput")
    mask_d = nc.dram_tensor("mask", [128, 128], bf16, kind="ExternalInput")
    out_d = nc.dram_tensor("out", [T, C], f32, kind="ExternalOutput")

    with tile.TileContext(nc) as tc:
        with tc.tile_pool(name="persist", bufs=1) as persist, \
             tc.tile_pool(name="xp", bufs=2) as xp, \
             tc.tile_pool(name="pt", bufs=6) as ptpool, \
             tc.tile_pool(name="ysb", bufs=4) as ypool, \
             tc.tile_pool(name="outst", bufs=2) as opool, \
             tc.tile_pool(name="rst", bufs=2) as rpool, \
             tc.tile_pool(name="psmm", bufs=2, space="PSUM") as psmm, \
             tc.tile_pool(name="psst", bufs=3, space="PSUM") as psst, \
             tc.tile_pool(name="psy", bufs=2, space="PSUM") as psy, \
             tc.tile_pool(name="psrep", bufs=1, space="PSUM") as psrep:

            with (tc.For_i(0, reps, 1) if reps > 1 else contextlib.nullcontext()):
                wqk_sb = persist.tile([128, 8, 2 * CL], bf16, tag="wqk")
                wv_sb = persist.tile([128, 8, CL], bf16, tag="wv")
                wp_sb = persist.tile([128, 4, C], bf16, tag="wp")
                qkT = persist.tile([128, 8, T], bf16, tag="qkT")
                V = persist.tile([128, NT, HPC, HD + 1], bf16, tag="V")
                yN = persist.tile([128, 4, T], bf16, tag="yN")
                bq_sb = persist.tile([128, 4], f32, tag="bq")
                mask_sb = persist.tile([128, 128], bf16, tag="mask")
                ones64 = persist.tile([1, 64], bf16, tag="ones")

                nc.sync.dma_start(bq_sb[:], bq_d[:])
                nc.sync.dma_start(mask_sb[:], mask_d[:])
                nc.sync.dma_start(
                    wqk_sb[:], wqk_d.rearrange("(j p) m -> p j m", p=128))
                nc.sync.dma_start(
                    wv_sb[:], wv_d.rearrange("(j p) m -> p j m", p=128))
                nc.sync.dma_start(
                    wp_sb[:], wp_d.rearrange("(mq p) oc -> p mq oc", p=128))
                nc.vector.memset(ones64[:], 1.0)
                nc.vector.memset(V[:, :, :, HD:HD + 1], 1.0)

                xq = [None] * NQ

                def load_x(c):
                    xq[c] = xp.tile([128, 8, 512], bf16, tag="x",
                                    name=f"x{c}")
                    nc.sync.dma_start(
                        xq[c][:],
                        xT_d[:, c * 512:c * 512 + 512]
                        .rearrange("(j p) n -> p j n", p=128))

                def qkv_part_ops(c, i):
                    """One quarter of chunk c's QKV as a list of closures,
                    each emitting one instruction (27 per part)."""
                    cs = c * 512
                    tb = c * 4 + i
                    pss = {}

                    def mk_qk(m, j):
                        def emit():
                            if j == 0:
                                pss[m] = psmm.tile([128, 512], f32,
                                                   tag="mm", name="mmqk")
                            nc.tensor.matmul(
                                pss[m][:],
                                lhsT=wqk_sb[:, j, m * 128:m * 128 + 128],
                                rhs=xq[c][:, j, :],
                                start=(j == 0), stop=(j == 7))
                        return emit

                    def mk_vmm(j):
                        def emit():
                            if j == 0:
                                pss['v'] = psmm.tile([128, 512], f32,
                                                     tag="mm", name="mmv")
                            nc.tensor.matmul(
                                pss['v'][:],
                                lhsT=xq[c][:, j, i * 128:i * 128 + 128],
                                rhs=wv_sb[:, j, :],
                                start=(j == 0), stop=(j == 7))
                        return emit

                    def mk_evac(m):
                        def emit():
                            if m == 'v':
                                nc.vector.tensor_copy(
                                    V[:, tb, :, 0:HD],
                                    pss['v'][:].rearrange(
                                        "p (h d) -> p h d", h=HPC))
                            elif m < 4:
                                nc.vector.tensor_scalar_add(
                                    qkT[:, m, cs:cs + 512], pss[m][:],
                                    bq_sb[:, m:m + 1])
                            else:
                                nc.vector.tensor_copy(
                                    qkT[:, m, cs:cs + 512], pss[m][:])
                        return emit

                    ops = []
                    for m in (i, i + 4):
                        ops += [mk_qk(m, j) for j in range(8)]
                        ops.append(mk_evac(m))
                    ops += [mk_vmm(j) for j in range(8)]
                    ops.append(mk_evac('v'))
                    return ops

                def proj_ops(c):
                    """Output projection for chunk c as closures (one
                    instruction each)."""
                    tiles = [(o, t) for o in range(2)
                             for t in range(c * 4, c * 4 + 4)]
                    pss = {}

                    def mk_mm(o, t, mq):
                        def emit():
                            if mq == 0:
                                pss[(o, t)] = psmm.tile([128, 512], f32,
                                                        tag="mm", name="mmpj")
                            nc.tensor.matmul(
                                pss[(o, t)][:],
                                lhsT=yN[:, mq, t * 128:t * 128 + 128],
                                rhs=wp_sb[:, mq, o * 512:o * 512 + 512],
                                start=(mq == 0), stop=(mq == 3))
                        return emit

                    def mk_evac(o, t):
                        def emit():
                            ost = opool.tile([128, 512], f32, tag="outst",
                                             name="ost")
                            nc.vector.tensor_copy(ost[:], pss[(o, t)][:])
                            nc.sync.dma_start(
                                out_d[t * 128:t * 128 + 128,
                                      o * 512:o * 512 + 512],
                                ost[:])
                        return emit

                    ops = []
                    for (o, t) in tiles:
                        ops += [mk_mm(o, t, mq) for mq in range(4)]
                        ops.append(mk_evac(o, t))
                    return ops

                def attn_pair(m, c, fill, sl, pending):
                    """Attention for head pair (2m, 2m+1), q-chunk c.
                    Pops filler ops per j-step (rate set so the fill list
                    drains evenly over the c-iteration) to fill PE gaps.
                    Deferred norm ops from the previous pair are emitted at
                    j==1, by which point their reciprocal inputs are long
                    done, so the rep matmuls don't stall the PE queue."""
                    cs = c * 512
                    nj = 4 * c + 4
                    ys = [psy.tile([65, 512], f32, tag="y",
                                   name=f"ys{m}{c}{s}") for s in (0, 1)]
                    prev = None
                    for j in range(nj):
                        jrel = j - 4 * c
                        z = 128 * jrel if jrel > 0 else 0
                        if j in (1, 2) and pending:
                            pending.pop(0)()
                        k = min(8, -(-len(fill) // max(sl[0], 1)))
                        sl[0] -= 1
                        for _ in range(k):
                            if fill:
                                fill.pop(0)()
                        pss = []
                        for s in (0, 1):
                            pb = 64 * s
                            ps_st = psst.tile([128, 512], f32, tag="st",
                                              name="st")
                            nc.tensor.matmul(
                                ps_st[:, z:512],
                                lhsT=qkT[pb:pb + 64, 4 + m,
                                         j * 128:j * 128 + 128],
                                rhs=qkT[pb:pb + 64, m, cs + z:cs + 512],
                                start=True, stop=True)
                            pss.append(ps_st)
                        pts = []
                        for s in (0, 1):
                            pt = ptpool.tile([128, 512], bf16, tag="pt",
                                             name="pt")
                            nc.scalar.activation(
                                pt[:, z:512], pss[s][:, z:512], Act.Exp)
                            pts.append(pt)
                        if jrel >= 0:
                            for s in (0, 1):
                                nc.gpsimd.tensor_tensor(
                                    pts[s][:, z:z + 128],
                                    pts[s][:, z:z + 128],
                                    mask_sb[:], Alu.mult)
                        if prev is not None:
                            pj, ppts, pz = prev
                            for s in (0, 1):
                                nc.tensor.matmul(
                                    ys[s][:, pz:512],
                                    lhsT=V[:, pj, 2 * m + s, :],
                                    rhs=ppts[s][:, pz:512],
                                    start=(pj == 0), stop=False)
                        prev = (j, pts, z)
                    pj, ppts, pz = prev
                    for s in (0, 1):
                        nc.tensor.matmul(
                            ys[s][:, pz:512],
                            lhsT=V[:, pj, 2 * m + s, :],
                            rhs=ppts[s][:, pz:512],
                            start=(pj == 0), stop=True)
                    ycs = []
                    for s in (0, 1):
                        y_c = ypool.tile([65, 512], f32, tag="ysb",
                                         name=f"y{m}{c}{s}")
                        nc.vector.tensor_copy(y_c[:], ys[s][:])
                        ycs.append(y_c)
                    return ycs

                def norm_pair(m, c, ycs):
                    """Reciprocals emitted eagerly (VectorE-only chain);
                    returns two closures (rep matmul + multiply per head)
                    to emit later, when the reciprocals are long done, so
                    the rep matmuls never stall the in-order PE queue."""
                    cs = c * 512
                    rTs = []
                    for s in (0, 1):
                        rT = rpool.tile([1, 512], bf16, tag="rT",
                                        name="rT")
                        with nc.allow_low_precision(
                                reason="bf16 1/denom; 0.4% scale noise ok"):
                            nc.vector.reciprocal(rT[:], ycs[s][64:65, :])
                        rTs.append(rT)

                    def mk(s):
                        def emit():
                            ps_rep = psrep.tile([64, 512], f32, tag="rep",
                                                name="rep")
                            nc.tensor.matmul(
                                ps_rep[:], lhsT=ones64[:],
                                rhs=rTs[s][:], start=True, stop=True)
                            nc.vector.tensor_tensor(
                                yN[64 * s:64 * s + 64, m, cs:cs + 512],
                                ycs[s][0:64, :], ps_rep[:], Alu.mult)
                        return emit
                    return [mk(0), mk(1)]

                # ---------------- schedule ----------------
                if "qkv" in phases:
                    load_x(0)
                    for op in [o for i in range(4)
                               for o in qkv_part_ops(0, i)]:
                        op()

                pending = []
                for c in range(NQ):
                    fill = []
                    if "qkv" in phases and c + 1 < NQ:
                        load_x(c + 1)
                        for i in range(4):
                            fill += qkv_part_ops(c + 1, i)
                    if "proj" in phases and c >= 1:
                        fill += proj_ops(c - 1)
                    if "attn" in phases:
                        sl = [4 * (4 * c + 4)]
                        for m in range(4):
                            ycs = attn_pair(m, c, fill, sl, pending)
                            if "norm" in phases:
                                pending += norm_pair(m, c, ycs)
                    for op in pending:
                        op() if c == NQ - 1 else None
                    if c == NQ - 1:
                        pending = []
                    for op in fill:
                        op()
                if "proj" in phases:
                    for op in proj_ops(NQ - 1):
                        op()

    nsplit = _split_multiwaits(nc)
    return nc, nsplit


def _prep_inputs(x, w_attn, b_attn, w_proj):
    """Per-core input maps. Core c: batch c//2, head-group c%2."""
    import ml_dtypes
    bf16 = ml_dtypes.bfloat16
    x = np.ascontiguousarray(x, dtype=np.float32)
    w_attn = np.asarray(w_attn, dtype=np.float32)
    b_attn = np.asarray(b_attn, dtype=np.float32)
    w_proj = np.asarray(w_proj, dtype=np.float32)
    scale = 1.0 / np.sqrt(HD)

    mask = (np.arange(128)[:, None] <= np.arange(128)[None, :]).astype(bf16)

    in_maps = []
    for core in range(NCORES):
        b = core // 2
        g = core % 2
        gc = CL * g
        wq = w_attn[gc:gc + CL, :] * scale          # [512, 1024]
        wk = w_attn[C + gc:C + gc + CL, :]
        wv = w_attn[2 * C + gc:2 * C + gc + CL, :]
        bq = b_attn[gc:gc + CL] * scale
        in_maps.append({
            "xT": np.ascontiguousarray(x[b].T.astype(bf16)),
            "wqk": np.ascontiguousarray(
                np.concatenate([wq.T, wk.T], axis=1).astype(bf16)),
            "wv": np.ascontiguousarray(wv.T.astype(bf16)),
            "bq": np.ascontiguousarray(bq.reshape(4, 128).T),
            "wp": np.ascontiguousarray(
                w_proj[:, gc:gc + CL].T.astype(bf16)),
            "mask": mask,
        })
    return in_maps


def _run(in_maps, reps=1):
    from concourse.bass_utils import run_bass_kernel_spmd
    key = reps
    if key not in _cache:
        _cache[key] = _build(reps)
    nc, _ = _cache[key]
    return run_bass_kernel_spmd(nc, in_maps, list(range(NCORES)))


def kernel(x, w_attn, b_attn, w_proj, b_proj):
    x = np.asarray(x, dtype=np.float32)
    w_attn = np.asarray(w_attn, dtype=np.float32)
    b_attn = np.asarray(b_attn, dtype=np.float32)
    w_proj = np.asarray(w_proj, dtype=np.float32)
    b_proj = np.asarray(b_proj, dtype=np.float32)

    in_maps = _prep_inputs(x, w_attn, b_attn, w_proj)
    res = _run(in_maps).results

    # host-side unshard: sum the two head-group partials per batch and add
    # the bias terms (b_proj + w_proj @ b_v; softmax rows sum to 1).
    bv = b_attn[2 * C:]
    const = (w_proj @ bv + b_proj).astype(np.float32)
    out = np.empty((B, T, C), dtype=np.float32)
    for b in range(B):
        out[b] = res[2 * b]["out"] + res[2 * b + 1]["out"] + const
    return out


# revision 13
# speedup vs baseline: 2.8458x; 2.2593x over previous
"""Causal self-attention (B=4, T=2048, C=1024, H=16) on 8 Trainium2 NeuronCores.

Sharding: 8 cores = 4 batches x 2 head-groups. Core c handles batch c//2 and
heads 8*(c%2) .. 8*(c%2)+8 (512 of the 1024 channels). Each core computes the
QKV projection for its channels over its batch's 2048 tokens, flash-style
causal attention for its 8 heads, and a partial output projection over its
512 c_proj input rows. The host sums the two partials per batch and adds the
bias terms (b_proj plus the b_v contribution, which is w_proj @ b_v because
softmax rows sum to one; b_k shifts every logit in a row equally so softmax
drops it; b_q and the 1/sqrt(hd) scale are folded into the q weights host-side).

All matmul inputs are bf16 (PSUM accumulation stays fp32). The schedule is
chunk-pipelined and latency-oriented: attention is ScalarE(exp)-bound per
tile, so the per-chunk loop emits, for each of the 4 head pairs, the j-loop
[S-pair row-tiled at partition bases 0/64, exp pair, diagonal mask on Pool,
PV pair one step behind] with QKV matmuls for the NEXT chunk and proj matmuls
for the PREVIOUS chunk hand-interleaved between j-steps to fill the PE gaps
left while ScalarE catches up. Band tiles (partially-masked diagonal blocks)
compute exp and the PV matmul only on the live suffix [z:512]; the masked
prefix is never written or read, so no zeroing and no persistent band
buffers are needed.

Softmax denominators ride in V's 65th column (ones), so ys row 64 is the
denominator; a [1,512] VectorE reciprocal and a K=1 ones-outer-product
matmul broadcast 1/denom across the 64 head channels for the normalize.

This container's walrus accepts only one hardware wait slot per instruction,
so after Tile scheduling we split multi-wait sync_info into standalone
EventSemaphore waits (_split_multiwaits).
"""

import sys

if '/opt/trn_rl_repo' not in sys.path:
    sys.path.insert(0, '/opt/trn_rl_repo')

import numpy as np

B, T, C, H = 4, 2048, 1024, 16
HD = C // H            # 64
HPC = 8                # heads per core
CL = HPC * HD          # 512 local channels
NCORES = 8
NQ = T // 512          # 4 q-chunks of 512 tokens
NT = T // 128          # 16 token-blocks

_cache = {}


def _split_multiwaits(nc, max_waits=1):
    import concourse.mybir as mybir
    n = 0
    ctr = [0]
    for fn in nc.m.functions:
        for bb in fn.blocks:
            out = []
            for inst in bb.instructions:
                si = inst.sync_info
                if si is not None and si.on_wait and len(si.on_wait) > max_waits:
                    waits = list(si.on_wait)
                    head, tail = waits[:-max_waits], waits[-max_waits:]
                    for w in head:
                        ctr[0] += 1
                        out.append(mybir.InstEventSemaphore(
                            name=f"wsplit-{ctr[0]}",
                            engine=inst.engine,
                            ins=[], outs=[],
                            sync_info=mybir.SyncInfo(on_wait=[w], on_update=[]),
                        ))
                    inst.sync_info = mybir.SyncInfo(
                        on_wait=tail, on_update=list(si.on_update))
                    n += 1
                out.append(inst)
            bb.instructions[:] = out
    return n


def _build(reps=1, phases="qkv,attn,norm,proj"):
    phases = set(phases.split(","))
    import contextlib
    import concourse.bass as bass
    import concourse.mybir as mybir
    import concourse.tile as tile

    f32 = mybir.dt.float32
    f32r = mybir.dt.float32r
    bf16 = mybir.dt.bfloat16
    Act = mybir.ActivationFunctionType
    Alu = mybir.AluOpType

    nc = bass.Bass()

    xT_d = nc.dram_tensor("xT", [C, T], bf16, kind="ExternalInput")
    wqk_d = nc.dram_tensor("wqk", [C, 2 * CL], bf16, kind="ExternalInput")
    wv_d = nc.dram_tensor("wv", [C, CL], bf16, kind="ExternalInput")
    bq_d = nc.dram_tensor("bq", [128, 4], f32, kind="ExternalInput")
    wp_d = nc.dram_tensor("wp", [CL, C], bf16, kind="ExternalInput")
    mask_d = nc.dram_tensor("mask", [128, 128], bf16, kind="ExternalInput")
    out_d = nc.dram_tensor("out", [T, C], f32, kind="ExternalOutput")

    with tile.TileContext(nc) as tc:
        with tc.tile_pool(name="persist", bufs=1) as persist, \
             tc.tile_pool(name="xp", bufs=2) as xp, \
             tc.tile_pool(name="pt", bufs=6) as ptpool, \
             tc.tile_pool(name="ysb", bufs=4) as ypool, \
             tc.tile_pool(name="outst", bufs=2) as opool, \
             tc.tile_pool(name="rst", bufs=2) as rpool, \
             tc.tile_pool(name="psmm", bufs=2, space="PSUM") as psmm, \
             tc.tile_pool(name="psst", bufs=3, space="PSUM") as psst, \
             tc.tile_pool(name="psy", bufs=2, space="PSUM") as psy, \
             tc.tile_pool(name="psrep", bufs=1, space="PSUM") as psrep:

            with (tc.For_i(0, reps, 1) if reps > 1 else contextlib.nullcontext()):
                wqk_sb = persist.tile([128, 8, 2 * CL], bf16, tag="wqk")
                wv_sb = persist.tile([128, 8, CL], bf16, tag="wv")
                wp_sb = persist.tile([128, 4, C], bf16, tag="wp")
                qkT = persist.tile([128, 8, T], bf16, tag="qkT")
                V = persist.tile([128, NT, HPC, HD + 1], bf16, tag="V")
                yN = persist.tile([128, 4, T], bf16, tag="yN")
                bq_sb = persist.tile([128, 4], f32, tag="bq")
                mask_sb = persist.tile([128, 128], bf16, tag="mask")
                ones64 = persist.tile([1, 64], bf16, tag="ones")

                nc.sync.dma_start(bq_sb[:], bq_d[:])
                nc.sync.dma_start(mask_sb[:], mask_d[:])
                nc.sync.dma_start(
                    wqk_sb[:], wqk_d.rearrange("(j p) m -> p j m", p=128))
                nc.sync.dma_start(
                    wv_sb[:], wv_d.rearrange("(j p) m -> p j m", p=128))
                nc.sync.dma_start(
                    wp_sb[:], wp_d.rearrange("(mq p) oc -> p mq oc", p=128))
                nc.vector.memset(ones64[:], 1.0)
                nc.vector.memset(V[:, :, :, HD:HD + 1], 1.0)

                xq = [None] * NQ

                def load_x(c):
                    xq[c] = xp.tile([128, 8, 512], bf16, tag="x",
                                    name=f"x{c}")
                    nc.sync.dma_start(
                        xq[c][:],
                        xT_d[:, c * 512:c * 512 + 512]
                        .rearrange("(j p) n -> p j n", p=128))

                def qkv_part_ops(c, i):
                    """One quarter of chunk c's QKV as a list of closures,
                    each emitting one instruction (27 per part)."""
                    cs = c * 512
                    tb = c * 4 + i
                    pss = {}

                    def mk_qk(m, j):
                        def emit():
                            if j == 0:
                                pss[m] = psmm.tile([128, 512], f32,
                                                   tag="mm", name="mmqk")
                            nc.tensor.matmul(
                                pss[m][:],
                                lhsT=wqk_sb[:, j, m * 128:m * 128 + 128],
                                rhs=xq[c][:, j, :],
                                start=(j == 0), stop=(j == 7))
                        return emit

                    def mk_vmm(j):
                        def emit():
                            if j == 0:
                                pss['v'] = psmm.tile([128, 512], f32,
                                                     tag="mm", name="mmv")
                            nc.tensor.matmul(
                                pss['v'][:],
                                lhsT=xq[c][:, j, i * 128:i * 128 + 128],
                                rhs=wv_sb[:, j, :],
                                start=(j == 0), stop=(j == 7))
                        return emit

                    def mk_evac(m):
                        def emit():
                            if m == 'v':
                                nc.vector.tensor_copy(
                                    V[:, tb, :, 0:HD],
                                    pss['v'][:].rearrange(
                                        "p (h d) -> p h d", h=HPC))
                            elif m < 4:
                                nc.vector.tensor_scalar_add(
                                    qkT[:, m, cs:cs + 512], pss[m][:],
                                    bq_sb[:, m:m + 1])
                            else:
                                nc.vector.tensor_copy(
                                    qkT[:, m, cs:cs + 512], pss[m][:])
                        return emit

                    ops = []
                    for m in (i, i + 4):
                        ops += [mk_qk(m, j) for j in range(8)]
                        ops.append(mk_evac(m))
                    ops += [mk_vmm(j) for j in range(8)]
                    ops.append(mk_evac('v'))
                    return ops

                def proj_ops(c):
                    """Output projection for chunk c as closures (one
                    instruction each)."""
                    tiles = [(o, t) for o in range(2)
                             for t in range(c * 4, c * 4 + 4)]
                    pss = {}

                    def mk_mm(o, t, mq):
                        def emit():
                            if mq == 0:
                                pss[(o, t)] = psmm.tile([128, 512], f32,
                                                        tag="mm", name="mmpj")
                            nc.tensor.matmul(
                                pss[(o, t)][:],
                                lhsT=yN[:, mq, t * 128:t * 128 + 128],
                                rhs=wp_sb[:, mq, o * 512:o * 512 + 512],
                                start=(mq == 0), stop=(mq == 3))
                        return emit

                    def mk_evac(o, t):
                        def emit():
                            ost = opool.tile([128, 512], f32, tag="outst",
                                             name="ost")
                            nc.vector.tensor_copy(ost[:], pss[(o, t)][:])
                            nc.sync.dma_start(
                                out_d[t * 128:t * 128 + 128,
                                      o * 512:o * 512 + 512],
                                ost[:])
                        return emit

                    ops = []
                    for (o, t) in tiles:
                        ops += [mk_mm(o, t, mq) for mq in range(4)]
                        ops.append(mk_evac(o, t))
                    return ops

                def attn_pair(m, c, fill, sl, pending):
                    """Attention for head pair (2m, 2m+1), q-chunk c.
                    Pops filler ops per j-step (rate set so the fill list
                    drains evenly over the c-iteration) to fill PE gaps.
                    Deferred norm ops from the previous pair are emitted at
                    j==1, by which point their reciprocal inputs are long
                    done, so the rep matmuls don't stall the PE queue."""
                    cs = c * 512
                    nj = 4 * c + 4
                    ys = [psy.tile([65, 512], f32, tag="y",
                                   name=f"ys{m}{c}{s}") for s in (0, 1)]
                    prev = None
                    for j in range(nj):
                        jrel = j - 4 * c
                        z = 128 * jrel if jrel > 0 else 0
                        if j in (1, 2) and pending:
                            pending.pop(0)()
                        k = min(8, -(-len(fill) // max(sl[0], 1)))
                        sl[0] -= 1
                        for _ in range(k):
                            if fill:
                                fill.pop(0)()
                        pss = []
                        for s in (0, 1):
                            pb = 64 * s
                            ps_st = psst.tile([128, 512], f32, tag="st",
                                              name="st")
                            nc.tensor.matmul(
                                ps_st[:, z:512],
                                lhsT=qkT[pb:pb + 64, 4 + m,
                                         j * 128:j * 128 + 128],
                                rhs=qkT[pb:pb + 64, m, cs + z:cs + 512],
                                start=True, stop=True)
                            pss.append(ps_st)
                        pts = []
                        for s in (0, 1):
                            pt = ptpool.tile([128, 512], bf16, tag="pt",
                                             name="pt")
                            nc.scalar.activation(
                                pt[:, z:512], pss[s][:, z:512], Act.Exp)
                            pts.append(pt)
                        if jrel >= 0:
                            for s in (0, 1):
                                nc.gpsimd.tensor_tensor(
                                    pts[s][:, z:z + 128],
                                    pts[s][:, z:z + 128],
                                    mask_sb[:], Alu.mult)
                        if prev is not None:
                            pj, ppts, pz = prev
                            for s in (0, 1):
                                nc.tensor.matmul(
                                    ys[s][:, pz:512],
                                    lhsT=V[:, pj, 2 * m + s, :],
                                    rhs=ppts[s][:, pz:512],
                                    start=(pj == 0), stop=False)
                        prev = (j, pts, z)
                    pj, ppts, pz = prev
                    for s in (0, 1):
                        nc.tensor.matmul(
                            ys[s][:, pz:512],
                            lhsT=V[:, pj, 2 * m + s, :],
                            rhs=ppts[s][:, pz:512],
                            start=(pj == 0), stop=True)
                    ycs = []
                    for s in (0, 1):
                        y_c = ypool.tile([65, 512], f32, tag="ysb",
                                         name=f"y{m}{c}{s}")
                        nc.vector.tensor_copy(y_c[:], ys[s][:])
                        ycs.append(y_c)
                    return ycs

                def norm_pair(m, c, ycs):
                    """Reciprocals emitted eagerly (VectorE-only chain);
                    returns two closures (rep matmul + multiply per head)
                    to emit later, when the reciprocals are long done, so
                    the rep matmuls never stall the in-order PE queue."""
                    cs = c * 512
                    rTs = []
                    for s in (0, 1):
                        rT = rpool.tile([1, 512], bf16, tag="rT",
                                        name="rT")
                        with nc.allow_low_precision(
                                reason="bf16 1/denom; 0.4% scale noise ok"):
                            nc.vector.reciprocal(rT[:], ycs[s][64:65, :])
                        rTs.append(rT)

                    def mk(s):
                        def emit():
                            ps_rep = psrep.tile([64, 512], f32, tag="rep",
                                                name="rep")
                            nc.tensor.matmul(
                                ps_rep[:], lhsT=ones64[:],
                                rhs=rTs[s][:], start=True, stop=True)
                            nc.vector.tensor_tensor(
                                yN[64 * s:64 * s + 64, m, cs:cs + 512],
                                ycs[s][0:64, :], ps_rep[:], Alu.mult)
                        return emit
                    return [mk(0), mk(1)]

                # ---------------- schedule ----------------
                if "qkv" in phases:
                    load_x(0)
                    for op in [o for i in range(4)
                               for o in qkv_part_ops(0, i)]:
                        op()

                pending = []
                for c in range(NQ):
                    fill = []
                    if "qkv" in phases and c + 1 < NQ:
                        load_x(c + 1)
                        for i in range(4):
                            fill += qkv_part_ops(c + 1, i)
                    if "proj" in phases and c >= 1:
                        fill += proj_ops(c - 1)
                    if "attn" in phases:
                        sl = [4 * (4 * c + 4)]
                        for m in range(4):
                            ycs = attn_pair(m, c, fill, sl, pending)
                            if "norm" in phases:
                                for op_ in norm_pair(m, c, ycs):
                                    op_()
                    for op in pending:
                        op() if c == NQ - 1 else None
                    if c == NQ - 1:
                        pending = []
                    for op in fill:
                        op()
                if "proj" in phases:
                    for op in proj_ops(NQ - 1):
                        op()

    nsplit = _split_multiwaits(nc)
    return nc, nsplit


def _prep_inputs(x, w_attn, b_attn, w_proj):
    """Per-core input maps. Core c: batch c//2, head-group c%2."""
    import ml_dtypes
    bf16 = ml_dtypes.bfloat16
    x = np.ascontiguousarray(x, dtype=np.float32)
    w_attn = np.asarray(w_attn, dtype=np.float32)
    b_attn = np.asarray(b_attn, dtype=np.float32)
    w_proj = np.asarray(w_proj, dtype=np.float32)
    scale = 1.0 / np.sqrt(HD)

    mask = (np.arange(128)[:, None] <= np.arange(128)[None, :]).astype(bf16)

    in_maps = []
    for core in range(NCORES):
        b = core // 2
        g = core % 2
        gc = CL * g
        wq = w_attn[gc:gc + CL, :] * scale          # [512, 1024]
        wk = w_attn[C + gc:C + gc + CL, :]
        wv = w_attn[2 * C + gc:2 * C + gc + CL, :]
        bq = b_attn[gc:gc + CL] * scale
        in_maps.append({
            "xT": np.ascontiguousarray(x[b].T.astype(bf16)),
            "wqk": np.ascontiguousarray(
                np.concatenate([wq.T, wk.T], axis=1).astype(bf16)),
            "wv": np.ascontiguousarray(wv.T.astype(bf16)),
            "bq": np.ascontiguousarray(bq.reshape(4, 128).T),
            "wp": np.ascontiguousarray(
                w_proj[:, gc:gc + CL].T.astype(bf16)),
            "mask": mask,
        })
    return in_maps


def _run(in_maps, reps=1):
    from concourse.bass_utils import run_bass_kernel_spmd
    key = reps
    if key not in _cache:
        _cache[key] = _build(reps)
    nc, _ = _cache[key]
    return run_bass_kernel_spmd(nc, in_maps, list(range(NCORES)))


def kernel(x, w_attn, b_attn, w_proj, b_proj):
    x = np.asarray(x, dtype=np.float32)
    w_attn = np.asarray(w_attn, dtype=np.float32)
    b_attn = np.asarray(b_attn, dtype=np.float32)
    w_proj = np.asarray(w_proj, dtype=np.float32)
    b_proj = np.asarray(b_proj, dtype=np.float32)

    in_maps = _prep_inputs(x, w_attn, b_attn, w_proj)
    res = _run(in_maps).results

    # host-side unshard: sum the two head-group partials per batch and add
    # the bias terms (b_proj + w_proj @ b_v; softmax rows sum to 1).
    bv = b_attn[2 * C:]
    const = (w_proj @ bv + b_proj).astype(np.float32)
    out = np.empty((B, T, C), dtype=np.float32)
    for b in range(B):
        out[b] = res[2 * b]["out"] + res[2 * b + 1]["out"] + const
    return out
